# revision 7
# baseline (speedup 1.0000x reference)
"""Trainium2 Bass kernel for the gated-cell module:

    rt = sigmoid(xt @ Wa.T + ba); it = sigmoid(xt @ Wx.T + bx)
    at = exp(-(C*softplus(Lambda)) * rt)
    ht = at * ht_minus_1 + sqrt(1 - at^2) * (it * xt)

Sharding: data-parallel over the batch dim across 8 NeuronCores; weights
replicated.  Compute runs in a transposed layout ([D, B] with D on the
partition axis) so the per-feature scale/bias vectors (ba, bx,
-C*softplus(Lambda)) ride in the ACT engine's per-partition scale/bias
operands for free, and the xt operand is already K-major for the PE.

sqrt(1-at^2) is computed as exp(0.5*ln(1-at^2)) so the whole post-sigmoid
chain stays in the single `natural_log_exp_and_others` ACT table set
(the hardware sqrt table has a 65536-ULP error budget; ln/exp are tight).
All ACT instructions are chained in emission order (sync=False deps) so
the scheduler cannot interleave sigmoid-set and exp-set ops — otherwise
every alternation costs a ~1.3us ACT_TABLE_LOAD.
"""

import sys

if "/opt/trn_rl_repo" not in sys.path:
    sys.path.insert(0, "/opt/trn_rl_repo")

import numpy as np

B, D = 16384, 1024
C = 8.0
NCORES = 8
BS = B // NCORES          # 2048 batch rows per core
PT = 128                  # partition tile
KT = D // PT              # 8 k-tiles (contraction)
JT = D // PT              # 8 j-tiles (output features)
CHUNKS = (1024, 1024)     # batch-chunk widths per core (sum == BS)

_CACHE = {}


def _build():
    from contextlib import ExitStack

    import concourse.mybir as mybir
    import concourse.tile as tile
    from concourse.tile import add_dep_helper
    from concourse import bacc

    f32 = mybir.dt.float32
    f32r = mybir.dt.float32r
    AF = mybir.ActivationFunctionType

    nc = bacc.Bacc("TRN2", target_bir_lowering=False, debug=False,
                   num_devices=NCORES)

    xtT = nc.dram_tensor("xtT", [D, BS], f32r, kind="ExternalInput").ap()
    htT = nc.dram_tensor("htT", [D, BS], f32, kind="ExternalInput").ap()
    waT = nc.dram_tensor("waT", [D, D], f32r, kind="ExternalInput").ap()
    wxT = nc.dram_tensor("wxT", [D, D], f32r, kind="ExternalInput").ap()
    biasA = nc.dram_tensor("biasA", [PT, JT], f32, kind="ExternalInput").ap()
    biasX = nc.dram_tensor("biasX", [PT, JT], f32, kind="ExternalInput").ap()
    negk = nc.dram_tensor("negk", [PT, JT], f32, kind="ExternalInput").ap()
    outT = nc.dram_tensor("outT", [D, BS], f32, kind="ExternalOutput").ap()

    act_chain = []

    def act(*args, **kwargs):
        inst = nc.scalar.activation(*args, **kwargs)
        act_chain.append(inst)
        return inst

    with tile.TileContext(nc) as tc, ExitStack() as ctx:
        wpool = ctx.enter_context(tc.tile_pool(name="w", bufs=1))
        cpool = ctx.enter_context(tc.tile_pool(name="c", bufs=1))
        xpool = ctx.enter_context(tc.tile_pool(name="x", bufs=1))
        gpool = ctx.enter_context(tc.tile_pool(name="g", bufs=1))
        tpool = ctx.enter_context(tc.tile_pool(name="t", bufs=2))
        pzpool = ctx.enter_context(tc.tile_pool(name="pz", bufs=2, space="PSUM"))

        biasA_sb = cpool.tile([PT, JT], f32, tag="ba")
        biasX_sb = cpool.tile([PT, JT], f32, tag="bx")
        negk_sb = cpool.tile([PT, JT], f32, tag="nk")
        nc.sync.dma_start(out=biasA_sb, in_=biasA)
        nc.sync.dma_start(out=biasX_sb, in_=biasX)
        nc.sync.dma_start(out=negk_sb, in_=negk)

        # First-chunk x tiles and weights, interleaved per k so the j=0
        # matmul accumulation can start after ~1.5MB of DMA instead of 12MB.
        Q0 = CHUNKS[0]
        x_sb0 = [xpool.tile([PT, Q0], f32r, name=f"xc0k{k}", tag=f"x{k}")
                 for k in range(KT)]
        wa_sb = [wpool.tile([PT, D], f32r, name=f"wa{k}", tag=f"wa{k}")
                 for k in range(KT)]
        wx_sb = [wpool.tile([PT, D], f32r, name=f"wx{k}", tag=f"wx{k}")
                 for k in range(KT)]
        for k in range(KT):
            nc.sync.dma_start(out=x_sb0[k], in_=xtT[k * PT:(k + 1) * PT, 0:Q0])
            nc.sync.dma_start(out=wa_sb[k], in_=waT[k * PT:(k + 1) * PT, :])
            nc.sync.dma_start(out=wx_sb[k], in_=wxT[k * PT:(k + 1) * PT, :])

        coff = 0
        for ci, Q in enumerate(CHUNKS):
            bsl = slice(coff, coff + Q)
            coff += Q
            NH = Q // 512

            if ci == 0:
                x_sb = x_sb0
            else:
                x_sb = [xpool.tile([PT, Q], f32r, name=f"xc{ci}k{k}",
                                   tag=f"x{k}") for k in range(KT)]
                for k in range(KT):
                    nc.sync.dma_start(out=x_sb[k],
                                      in_=xtT[k * PT:(k + 1) * PT, bsl])

            rt_g = gpool.tile([PT, JT, Q], f32, tag="rt", name=f"rt{ci}")
            p_g = gpool.tile([PT, JT, Q], f32, tag="p", name=f"p{ci}")

            # ---- phase 1: GEMMs (fp32r) + sigmoids; p = it * xt ----
            for j in range(JT):
                jsl = slice(j * PT, (j + 1) * PT)
                za = pzpool.tile([PT, Q], f32, tag="za", name=f"za{ci}_{j}")
                zx = pzpool.tile([PT, Q], f32, tag="zx", name=f"zx{ci}_{j}")
                for k in range(KT):
                    for h in range(NH):
                        nsl = slice(h * 512, (h + 1) * 512)
                        nc.tensor.matmul(za[:, nsl], wa_sb[k][:, jsl],
                                         x_sb[k][:, nsl],
                                         start=(k == 0), stop=(k == KT - 1))
                for k in range(KT):
                    for h in range(NH):
                        nsl = slice(h * 512, (h + 1) * 512)
                        nc.tensor.matmul(zx[:, nsl], wx_sb[k][:, jsl],
                                         x_sb[k][:, nsl],
                                         start=(k == 0), stop=(k == KT - 1))
                act(out=rt_g[:, j, :], in_=za, func=AF.Sigmoid,
                    bias=biasA_sb[:, j:j + 1], scale=1.0)
                act(out=p_g[:, j, :], in_=zx, func=AF.Sigmoid,
                    bias=biasX_sb[:, j:j + 1], scale=1.0)
                nc.vector.tensor_mul(p_g[:, j, :], p_g[:, j, :],
                                     x_sb[j].bitcast(f32))

            # ---- phase 2: at, sqrt(1-at^2) via ln/exp, combine, store ----
            for j in range(JT):
                jsl = slice(j * PT, (j + 1) * PT)
                h_t = tpool.tile([PT, Q], f32, tag="h", name=f"h{ci}_{j}")
                nc.sync.dma_start(out=h_t, in_=htT[jsl, bsl])

                at_t = tpool.tile([PT, Q], f32, tag="at", name=f"at{ci}_{j}")
                act(out=at_t, in_=rt_g[:, j, :], func=AF.Exp,
                    scale=negk_sb[:, j:j + 1])
                a2 = tpool.tile([PT, Q], f32, tag="a2", name=f"a2{ci}_{j}")
                nc.vector.tensor_mul(a2, at_t, at_t)
                m1 = tpool.tile([PT, Q], f32, tag="m1", name=f"m1{ci}_{j}")
                nc.vector.tensor_mul(m1, at_t, h_t)
                # a2 <- ln(1 - a2), then a2 <- exp(0.5*ln) = sqrt(1-at^2)
                act(out=a2, in_=a2, func=AF.Ln, bias=1.0, scale=-1.0)
                act(out=a2, in_=a2, func=AF.Exp, scale=0.5)
                m3 = tpool.tile([PT, Q], f32, tag="m3", bufs=1,
                                name=f"m3{ci}_{j}")
                nc.vector.tensor_mul(m3, a2, p_g[:, j, :])
                o = tpool.tile([PT, Q], f32, tag="o", name=f"o{ci}_{j}")
                nc.vector.tensor_add(o, m1, m3)
                nc.sync.dma_start(out=outT[jsl, bsl], in_=o)

        # Pin the ACT stream to emission order: all sigmoids of a chunk,
        # then all exp/ln ops, then the next chunk's sigmoids.  This caps
        # the number of ACT table-set switches at 4 for the whole kernel.
        for a, b in zip(act_chain, act_chain[1:]):
            add_dep_helper(b.ins, a.ins, sync=False, reason="act set order")

    nc.compile()
    return nc


def _np_softplus(x):
    return np.logaddexp(0.0, x)


def _fold(vec):
    # [D] feature vector -> [128, JT] tile where column j holds features
    # j*128 .. j*128+127 (per-partition scalars for j-tile j).
    return np.ascontiguousarray(vec.reshape(JT, PT).T)


def kernel(xt, ht_minus_1, Wa, Wx, ba, bx, Lambda):
    from concourse.bass_utils import run_bass_kernel_spmd

    if "nc" not in _CACHE:
        _CACHE["nc"] = _build()
    nc = _CACHE["nc"]

    xt = np.asarray(xt, dtype=np.float32)
    ht = np.asarray(ht_minus_1, dtype=np.float32)
    Wa = np.asarray(Wa, dtype=np.float32)
    Wx = np.asarray(Wx, dtype=np.float32)
    ba = np.asarray(ba, dtype=np.float32).reshape(-1)
    bx = np.asarray(bx, dtype=np.float32).reshape(-1)
    Lam = np.asarray(Lambda, dtype=np.float32).reshape(-1)

    negk_vec = (-C * _np_softplus(Lam.astype(np.float64))).astype(np.float32)

    xtT = np.ascontiguousarray(xt.T)
    htT = np.ascontiguousarray(ht.T)
    waT = np.ascontiguousarray(Wa.T)
    wxT = np.ascontiguousarray(Wx.T)
    biasA = _fold(ba)
    biasX = _fold(bx)
    negk = _fold(negk_vec)

    in_maps = []
    for c in range(NCORES):
        sl = slice(c * BS, (c + 1) * BS)
        in_maps.append({
            "xtT": np.ascontiguousarray(xtT[:, sl]),
            "htT": np.ascontiguousarray(htT[:, sl]),
            "waT": waT,
            "wxT": wxT,
            "biasA": biasA,
            "biasX": biasX,
            "negk": negk,
        })

    res = run_bass_kernel_spmd(nc, in_maps, list(range(NCORES)))
    outT = np.concatenate([res.results[c]["outT"] for c in range(NCORES)],
                          axis=1)
    return np.ascontiguousarray(outT.T)


# revision 9
# speedup vs baseline: 1.1390x; 1.1390x over previous
"""Trainium2 Bass kernel for the gated-cell module:

    rt = sigmoid(xt @ Wa.T + ba); it = sigmoid(xt @ Wx.T + bx)
    at = exp(-(C*softplus(Lambda)) * rt)
    ht = at * ht_minus_1 + sqrt(1 - at^2) * (it * xt)

Sharding: data-parallel over the batch dim across 8 NeuronCores; weights
replicated.  Compute runs in a transposed layout ([D, B] with D on the
partition axis) so the per-feature vectors (ba, bx, -C*softplus(Lambda))
ride in the ACT engine's per-partition scale/bias operands, and xt is
already K-major for the PE.

Matmuls and element-wise intermediates run in bf16 (fp32 PSUM
accumulation, fp32 output): bf16 matmul streams at full PE rate while
fp32 runs at 1/4, and bf16 doubles DVE throughput.  sqrt(1-at^2) is
computed as exp(0.5*ln(1-at^2)) on wide group tiles; ACT instructions
are chained in emission order (sync=False deps) so the scheduler cannot
interleave different ACT table sets — every alternation would cost a
~1.5us ACT_TABLE_LOAD.
"""

import sys

if "/opt/trn_rl_repo" not in sys.path:
    sys.path.insert(0, "/opt/trn_rl_repo")

import numpy as np

B, D = 16384, 1024
C = 8.0
NCORES = 8
BS = B // NCORES          # 2048 batch rows per core
PT = 128                  # partition tile
KT = D // PT              # 8 k-tiles (contraction)
JT = D // PT              # 8 j-tiles (output features)
CHUNKS = (256, 896, 896)  # batch-chunk widths per core (sum == BS)
# chunk 0 is small so the PE can start after ~2.5MB of input DMA

_CACHE = {}


def _build():
    from contextlib import ExitStack

    import concourse.mybir as mybir
    import concourse.tile as tile
    from concourse.tile import add_dep_helper
    from concourse import bacc

    f32 = mybir.dt.float32
    bf16 = mybir.dt.bfloat16
    AF = mybir.ActivationFunctionType

    nc = bacc.Bacc("TRN2", target_bir_lowering=False, debug=False,
                   num_devices=NCORES, dynamic_dma_scratch_size=4096)

    xtT = nc.dram_tensor("xtT", [D, BS], bf16, kind="ExternalInput").ap()
    htT = nc.dram_tensor("htT", [D, BS], bf16, kind="ExternalInput").ap()
    waT = nc.dram_tensor("waT", [D, D], bf16, kind="ExternalInput").ap()
    wxT = nc.dram_tensor("wxT", [D, D], bf16, kind="ExternalInput").ap()
    biasA = nc.dram_tensor("biasA", [PT, JT], f32, kind="ExternalInput").ap()
    biasX = nc.dram_tensor("biasX", [PT, JT], f32, kind="ExternalInput").ap()
    negk = nc.dram_tensor("negk", [PT, JT], f32, kind="ExternalInput").ap()
    outT = nc.dram_tensor("outT", [D, BS], f32, kind="ExternalOutput").ap()

    act_chain = []

    def act(*args, **kwargs):
        inst = nc.scalar.activation(*args, **kwargs)
        act_chain.append(inst)
        return inst

    with tile.TileContext(nc) as tc, ExitStack() as ctx:
        wpool = ctx.enter_context(tc.tile_pool(name="w", bufs=1))
        cpool = ctx.enter_context(tc.tile_pool(name="c", bufs=1))
        xpool = ctx.enter_context(tc.tile_pool(name="x", bufs=2))
        gpool = ctx.enter_context(tc.tile_pool(name="g", bufs=2))
        tpool = ctx.enter_context(tc.tile_pool(name="t", bufs=3))
        pzpool = ctx.enter_context(tc.tile_pool(name="pz", bufs=2, space="PSUM"))

        biasA_sb = cpool.tile([PT, JT], f32, tag="ba")
        biasX_sb = cpool.tile([PT, JT], f32, tag="bx")
        negk_sb = cpool.tile([PT, JT], f32, tag="nk")
        nc.sync.dma_start(out=biasA_sb, in_=biasA)
        nc.sync.dma_start(out=biasX_sb, in_=biasX)
        nc.sync.dma_start(out=negk_sb, in_=negk)

        # DMA order: chunk-0 x first, then Wa, then Wx, so the first
        # accumulation group's operands land as early as possible.
        Q0 = CHUNKS[0]
        x_sb0 = [xpool.tile([PT, Q0], bf16, name=f"xc0k{k}", tag=f"x{k}")
                 for k in range(KT)]
        wa_sb = [wpool.tile([PT, D], bf16, name=f"wa{k}", tag=f"wa{k}")
                 for k in range(KT)]
        wx_sb = [wpool.tile([PT, D], bf16, name=f"wx{k}", tag=f"wx{k}")
                 for k in range(KT)]
        for k in range(KT):
            nc.sync.dma_start(out=x_sb0[k], in_=xtT[k * PT:(k + 1) * PT, 0:Q0])
        for k in range(KT):
            nc.sync.dma_start(out=wa_sb[k], in_=waT[k * PT:(k + 1) * PT, :])
        for k in range(KT):
            nc.sync.dma_start(out=wx_sb[k], in_=wxT[k * PT:(k + 1) * PT, :])

        coff = 0
        for ci, Q in enumerate(CHUNKS):
            bsl = slice(coff, coff + Q)
            coff += Q
            nsls = []
            off = 0
            while off < Q:
                w = min(512, Q - off)
                nsls.append(slice(off, off + w))
                off += w

            if ci == 0:
                x_sb = x_sb0
            else:
                x_sb = [xpool.tile([PT, Q], bf16, name=f"xc{ci}k{k}",
                                   tag=f"x{k}") for k in range(KT)]
                for k in range(KT):
                    nc.sync.dma_start(out=x_sb[k],
                                      in_=xtT[k * PT:(k + 1) * PT, bsl])

            rt_g = gpool.tile([PT, JT, Q], bf16, tag="rt", name=f"rt{ci}")
            p_g = gpool.tile([PT, JT, Q], bf16, tag="p", name=f"p{ci}")

            # ---- phase 1: GEMMs (bf16, fp32 PSUM) + sigmoids; p = it*xt ----
            for j in range(JT):
                jsl = slice(j * PT, (j + 1) * PT)
                za = pzpool.tile([PT, Q], f32, tag="za", name=f"za{ci}_{j}")
                zx = pzpool.tile([PT, Q], f32, tag="zx", name=f"zx{ci}_{j}")
                for k in range(KT):
                    for nsl in nsls:
                        nc.tensor.matmul(za[:, nsl], wa_sb[k][:, jsl],
                                         x_sb[k][:, nsl],
                                         start=(k == 0), stop=(k == KT - 1))
                for k in range(KT):
                    for nsl in nsls:
                        nc.tensor.matmul(zx[:, nsl], wx_sb[k][:, jsl],
                                         x_sb[k][:, nsl],
                                         start=(k == 0), stop=(k == KT - 1))
                act(out=rt_g[:, j, :], in_=za, func=AF.Sigmoid,
                    bias=biasA_sb[:, j:j + 1], scale=1.0)
                act(out=p_g[:, j, :], in_=zx, func=AF.Sigmoid,
                    bias=biasX_sb[:, j:j + 1], scale=1.0)
                nc.vector.tensor_mul(p_g[:, j, :], p_g[:, j, :], x_sb[j])

            # ---- phase 2 ----
            # rt <- negk*rt per j (DVE), then one wide Exp -> at = rt_g.
            for j in range(JT):
                nc.vector.tensor_scalar_mul(rt_g[:, j, :], rt_g[:, j, :],
                                            negk_sb[:, j:j + 1])
            act(out=rt_g, in_=rt_g, func=AF.Exp)

            a2_g = gpool.tile([PT, JT, Q], bf16, tag="a2", name=f"a2{ci}")
            m1_g = gpool.tile([PT, JT, Q], bf16, tag="m1", bufs=1,
                              name=f"m1{ci}")
            o_g = gpool.tile([PT, JT, Q], f32, tag="o", bufs=1,
                             name=f"o{ci}")
            for j in range(JT):
                jsl = slice(j * PT, (j + 1) * PT)
                h_t = tpool.tile([PT, Q], bf16, tag="h", name=f"h{ci}_{j}")
                nc.sync.dma_start(out=h_t, in_=htT[jsl, bsl])
                nc.vector.tensor_mul(a2_g[:, j, :], rt_g[:, j, :],
                                     rt_g[:, j, :])
                nc.vector.tensor_mul(m1_g[:, j, :], rt_g[:, j, :], h_t)
            # a2 <- ln(1 - a2) ; a2 <- exp(0.5*ln) = sqrt(1 - at^2)
            act(out=a2_g, in_=a2_g, func=AF.Ln, bias=1.0, scale=-1.0)
            act(out=a2_g, in_=a2_g, func=AF.Exp, scale=0.5)
            for j in range(JT):
                jsl = slice(j * PT, (j + 1) * PT)
                nc.vector.tensor_mul(p_g[:, j, :], a2_g[:, j, :],
                                     p_g[:, j, :])
                nc.vector.tensor_add(o_g[:, j, :], m1_g[:, j, :],
                                     p_g[:, j, :])
                nc.sync.dma_start(out=outT[jsl, bsl], in_=o_g[:, j, :])

        # Pin the ACT stream to emission order; caps table-set switches.
        for a, b in zip(act_chain, act_chain[1:]):
            add_dep_helper(b.ins, a.ins, sync=False, reason="act set order")

    nc.compile()
    return nc


def _np_softplus(x):
    return np.logaddexp(0.0, x)


def _fold(vec):
    # [D] feature vector -> [128, JT] tile where column j holds features
    # j*128 .. j*128+127 (per-partition scalars for j-tile j).
    return np.ascontiguousarray(vec.reshape(JT, PT).T)


def _prep(xt, ht, Wa, Wx, ba, bx, Lam):
    import ml_dtypes

    bf16 = ml_dtypes.bfloat16
    negk_vec = (-C * _np_softplus(Lam.astype(np.float64))).astype(np.float32)
    xtT = np.ascontiguousarray(xt.T.astype(bf16))
    htT = np.ascontiguousarray(ht.T.astype(bf16))
    waT = np.ascontiguousarray(Wa.T.astype(bf16))
    wxT = np.ascontiguousarray(Wx.T.astype(bf16))
    biasA = _fold(ba)
    biasX = _fold(bx)
    negk = _fold(negk_vec)
    in_maps = []
    for c in range(NCORES):
        sl = slice(c * BS, (c + 1) * BS)
        in_maps.append({
            "xtT": np.ascontiguousarray(xtT[:, sl]),
            "htT": np.ascontiguousarray(htT[:, sl]),
            "waT": waT,
            "wxT": wxT,
            "biasA": biasA,
            "biasX": biasX,
            "negk": negk,
        })
    return in_maps


def kernel(xt, ht_minus_1, Wa, Wx, ba, bx, Lambda):
    from concourse.bass_utils import run_bass_kernel_spmd

    if "nc" not in _CACHE:
        _CACHE["nc"] = _build()
    nc = _CACHE["nc"]

    in_maps = _prep(
        np.asarray(xt, dtype=np.float32),
        np.asarray(ht_minus_1, dtype=np.float32),
        np.asarray(Wa, dtype=np.float32),
        np.asarray(Wx, dtype=np.float32),
        np.asarray(ba, dtype=np.float32).reshape(-1),
        np.asarray(bx, dtype=np.float32).reshape(-1),
        np.asarray(Lambda, dtype=np.float32).reshape(-1),
    )
    res = run_bass_kernel_spmd(nc, in_maps, list(range(NCORES)))
    outT = np.concatenate([res.results[c]["outT"] for c in range(NCORES)],
                          axis=1)
    return np.ascontiguousarray(outT.T)


# revision 10
# speedup vs baseline: 1.2533x; 1.1004x over previous
"""Trainium2 Bass kernel for the gated-cell module:

    rt = sigmoid(xt @ Wa.T + ba); it = sigmoid(xt @ Wx.T + bx)
    at = exp(-(C*softplus(Lambda)) * rt)
    ht = at * ht_minus_1 + sqrt(1 - at^2) * (it * xt)

Sharding: data-parallel over the batch dim across 8 NeuronCores; weights
replicated.  Compute runs in a transposed layout ([D, B] with D on the
partition axis) so the per-feature vectors (ba, bx, -C*softplus(Lambda))
ride in the ACT engine's per-partition scale/bias operands, and xt is
already K-major for the PE.

Matmuls and element-wise intermediates run in bf16 (fp32 PSUM
accumulation, fp32 output): bf16 matmul streams at full PE rate while
fp32 runs at 1/4, and bf16 doubles DVE throughput.  sqrt(1-at^2) is
computed as exp(0.5*ln(1-at^2)) on wide group tiles; ACT instructions
are chained in emission order (sync=False deps) so the scheduler cannot
interleave different ACT table sets — every alternation would cost a
~1.5us ACT_TABLE_LOAD.
"""

import sys

if "/opt/trn_rl_repo" not in sys.path:
    sys.path.insert(0, "/opt/trn_rl_repo")

import numpy as np

B, D = 16384, 1024
C = 8.0
NCORES = 8
BS = B // NCORES          # 2048 batch rows per core
PT = 128                  # partition tile
KT = D // PT              # 8 k-tiles (contraction)
JT = D // PT              # 8 j-tiles (output features)
CHUNKS = (256, 896, 896)  # batch-chunk widths per core (sum == BS)
# chunk 0 is small so the PE can start after ~2.5MB of input DMA

_CACHE = {}


def _build():
    from contextlib import ExitStack

    import concourse.mybir as mybir
    import concourse.tile as tile
    from concourse.tile import add_dep_helper
    from concourse import bacc

    f32 = mybir.dt.float32
    bf16 = mybir.dt.bfloat16
    AF = mybir.ActivationFunctionType

    nc = bacc.Bacc("TRN2", target_bir_lowering=False, debug=False,
                   num_devices=NCORES, dynamic_dma_scratch_size=4096)

    xtT = nc.dram_tensor("xtT", [D, BS], bf16, kind="ExternalInput").ap()
    htT = nc.dram_tensor("htT", [D, BS], bf16, kind="ExternalInput").ap()
    waT = nc.dram_tensor("waT", [D, D], bf16, kind="ExternalInput").ap()
    wxT = nc.dram_tensor("wxT", [D, D], bf16, kind="ExternalInput").ap()
    biasA = nc.dram_tensor("biasA", [PT, JT], f32, kind="ExternalInput").ap()
    biasX = nc.dram_tensor("biasX", [PT, JT], f32, kind="ExternalInput").ap()
    negk = nc.dram_tensor("negk", [PT, JT], f32, kind="ExternalInput").ap()
    outT = nc.dram_tensor("outT", [D, BS], f32, kind="ExternalOutput").ap()

    # ACT instructions per chunk, gathered to build an explicit ordering
    # chain that keeps same-table-set ops contiguous.
    sig_ops = [[] for _ in CHUNKS]   # sigmoid-set ops (phase 1)
    ph2_ops = [[] for _ in CHUNKS]   # exp/ln-set ops (phase 2)

    with tile.TileContext(nc) as tc, ExitStack() as ctx:
        wpool = ctx.enter_context(tc.tile_pool(name="w", bufs=1))
        cpool = ctx.enter_context(tc.tile_pool(name="c", bufs=1))
        xpool = ctx.enter_context(tc.tile_pool(name="x", bufs=2))
        gpool = ctx.enter_context(tc.tile_pool(name="g", bufs=2))
        tpool = ctx.enter_context(tc.tile_pool(name="t", bufs=1))
        pzpool = ctx.enter_context(tc.tile_pool(name="pz", bufs=2, space="PSUM"))

        biasA_sb = cpool.tile([PT, JT], f32, tag="ba")
        biasX_sb = cpool.tile([PT, JT], f32, tag="bx")
        negk_sb = cpool.tile([PT, JT], f32, tag="nk")
        nc.sync.dma_start(out=biasA_sb, in_=biasA)
        nc.sync.dma_start(out=biasX_sb, in_=biasX)
        nc.sync.dma_start(out=negk_sb, in_=negk)

        # Chunk-0 x first, then Wa, then Wx: the first accumulation group
        # can start after ~2.5MB of DMA.
        Q0 = CHUNKS[0]
        x_g0 = xpool.tile([PT, KT, Q0], bf16, name="xg0", tag="x")
        nc.sync.dma_start(out=x_g0,
                          in_=xtT[:, 0:Q0].rearrange("(kt p) q -> p kt q", p=PT))
        wa_g = wpool.tile([PT, KT, D], bf16, name="wag", tag="wa")
        nc.sync.dma_start(out=wa_g,
                          in_=waT.rearrange("(kt p) j -> p kt j", p=PT))
        wx_g = wpool.tile([PT, KT, D], bf16, name="wxg", tag="wx")
        nc.sync.dma_start(out=wx_g,
                          in_=wxT.rearrange("(kt p) j -> p kt j", p=PT))

        coff = 0
        for ci, Q in enumerate(CHUNKS):
            bsl = slice(coff, coff + Q)
            coff += Q
            nsls = []
            off = 0
            while off < Q:
                w = min(512, Q - off)
                nsls.append(slice(off, off + w))
                off += w

            if ci == 0:
                x_g = x_g0
            else:
                x_g = xpool.tile([PT, KT, Q], bf16, name=f"xg{ci}", tag="x")
                nc.sync.dma_start(
                    out=x_g,
                    in_=xtT[:, bsl].rearrange("(kt p) q -> p kt q", p=PT))

            rt_g = gpool.tile([PT, JT, Q], bf16, tag="rt", name=f"rt{ci}")
            p_g = gpool.tile([PT, JT, Q], bf16, tag="p", name=f"p{ci}")

            # ---- phase 1: GEMMs (bf16, fp32 PSUM) + sigmoids; p = it*xt ----
            for j in range(JT):
                jsl = slice(j * PT, (j + 1) * PT)
                za = pzpool.tile([PT, Q], f32, tag="za", name=f"za{ci}_{j}")
                zx = pzpool.tile([PT, Q], f32, tag="zx", name=f"zx{ci}_{j}")
                for k in range(KT):
                    for nsl in nsls:
                        nc.tensor.matmul(za[:, nsl], wa_g[:, k, jsl],
                                         x_g[:, k, nsl],
                                         start=(k == 0), stop=(k == KT - 1))
                for k in range(KT):
                    for nsl in nsls:
                        nc.tensor.matmul(zx[:, nsl], wx_g[:, k, jsl],
                                         x_g[:, k, nsl],
                                         start=(k == 0), stop=(k == KT - 1))
                sig_ops[ci].append(
                    nc.scalar.activation(out=rt_g[:, j, :], in_=za,
                                         func=AF.Sigmoid,
                                         bias=biasA_sb[:, j:j + 1], scale=1.0))
                sig_ops[ci].append(
                    nc.scalar.activation(out=p_g[:, j, :], in_=zx,
                                         func=AF.Sigmoid,
                                         bias=biasX_sb[:, j:j + 1], scale=1.0))
                nc.vector.tensor_mul(p_g[:, j, :], p_g[:, j, :], x_g[:, j, :])

            # ---- phase 2, in two half-group waves per ACT pass ----
            for j in range(JT):
                nc.vector.tensor_scalar_mul(rt_g[:, j, :], rt_g[:, j, :],
                                            negk_sb[:, j:j + 1])

            h_g = tpool.tile([PT, JT, Q], bf16, tag="h", name=f"h{ci}")
            halves = [slice(0, JT // 2), slice(JT // 2, JT)]
            for hs in halves:
                nc.sync.dma_start(
                    out=h_g[:, hs, :],
                    in_=htT[hs.start * PT:hs.stop * PT, bsl].rearrange(
                        "(jt p) q -> p jt q", p=PT))

            a2_g = gpool.tile([PT, JT, Q], bf16, tag="a2", name=f"a2{ci}")
            m1_g = gpool.tile([PT, JT, Q], bf16, tag="m1", bufs=1,
                              name=f"m1{ci}")
            o_g = gpool.tile([PT, JT, Q], f32, tag="o", bufs=1, name=f"o{ci}")

            for hs in halves:
                ph2_ops[ci].append(
                    nc.scalar.activation(out=rt_g[:, hs, :],
                                         in_=rt_g[:, hs, :], func=AF.Exp))
            for j in range(JT):
                nc.vector.tensor_mul(a2_g[:, j, :], rt_g[:, j, :],
                                     rt_g[:, j, :])
                nc.vector.tensor_mul(m1_g[:, j, :], rt_g[:, j, :],
                                     h_g[:, j, :])
            for hs in halves:
                ph2_ops[ci].append(
                    nc.scalar.activation(out=a2_g[:, hs, :],
                                         in_=a2_g[:, hs, :], func=AF.Ln,
                                         bias=1.0, scale=-1.0))
            for hs in halves:
                ph2_ops[ci].append(
                    nc.scalar.activation(out=a2_g[:, hs, :],
                                         in_=a2_g[:, hs, :], func=AF.Exp,
                                         scale=0.5))
            for hs in halves:
                for j in range(hs.start, hs.stop):
                    nc.vector.tensor_mul(p_g[:, j, :], a2_g[:, j, :],
                                         p_g[:, j, :])
                    nc.vector.tensor_add(o_g[:, j, :], m1_g[:, j, :],
                                         p_g[:, j, :])
                nc.sync.dma_start(
                    out=outT[hs.start * PT:hs.stop * PT, bsl].rearrange(
                        "(jt p) q -> p jt q", p=PT),
                    in_=o_g[:, hs, :])

        # ACT ordering: sigmoids of chunk c, then the first two j-groups of
        # chunk c+1's sigmoids (same table set — extends the PE's PSUM
        # runway past the phase-2 block), then chunk c's exp/ln ops.
        PULL = 4  # 2 j-groups * 2 sigmoids
        chain = []
        for ci in range(len(CHUNKS)):
            chain += sig_ops[ci][(PULL if ci > 0 else 0):]
            if ci + 1 < len(CHUNKS):
                chain += sig_ops[ci + 1][:PULL]
            chain += ph2_ops[ci]
        for a, b in zip(chain, chain[1:]):
            add_dep_helper(b.ins, a.ins, sync=False, reason="act set order")

    nc.compile()
    return nc


def _np_softplus(x):
    return np.logaddexp(0.0, x)


def _fold(vec):
    # [D] feature vector -> [128, JT] tile where column j holds features
    # j*128 .. j*128+127 (per-partition scalars for j-tile j).
    return np.ascontiguousarray(vec.reshape(JT, PT).T)


def _prep(xt, ht, Wa, Wx, ba, bx, Lam):
    import ml_dtypes

    bf16 = ml_dtypes.bfloat16
    negk_vec = (-C * _np_softplus(Lam.astype(np.float64))).astype(np.float32)
    xtT = np.ascontiguousarray(xt.T.astype(bf16))
    htT = np.ascontiguousarray(ht.T.astype(bf16))
    waT = np.ascontiguousarray(Wa.T.astype(bf16))
    wxT = np.ascontiguousarray(Wx.T.astype(bf16))
    biasA = _fold(ba)
    biasX = _fold(bx)
    negk = _fold(negk_vec)
    in_maps = []
    for c in range(NCORES):
        sl = slice(c * BS, (c + 1) * BS)
        in_maps.append({
            "xtT": np.ascontiguousarray(xtT[:, sl]),
            "htT": np.ascontiguousarray(htT[:, sl]),
            "waT": waT,
            "wxT": wxT,
            "biasA": biasA,
            "biasX": biasX,
            "negk": negk,
        })
    return in_maps


def kernel(xt, ht_minus_1, Wa, Wx, ba, bx, Lambda):
    from concourse.bass_utils import run_bass_kernel_spmd

    if "nc" not in _CACHE:
        _CACHE["nc"] = _build()
    nc = _CACHE["nc"]

    in_maps = _prep(
        np.asarray(xt, dtype=np.float32),
        np.asarray(ht_minus_1, dtype=np.float32),
        np.asarray(Wa, dtype=np.float32),
        np.asarray(Wx, dtype=np.float32),
        np.asarray(ba, dtype=np.float32).reshape(-1),
        np.asarray(bx, dtype=np.float32).reshape(-1),
        np.asarray(Lambda, dtype=np.float32).reshape(-1),
    )
    res = run_bass_kernel_spmd(nc, in_maps, list(range(NCORES)))
    outT = np.concatenate([res.results[c]["outT"] for c in range(NCORES)],
                          axis=1)
    return np.ascontiguousarray(outT.T)


# revision 14
# speedup vs baseline: 1.5002x; 1.1970x over previous
"""Trainium2 Bass kernel for the gated-cell module:

    rt = sigmoid(xt @ Wa.T + ba); it = sigmoid(xt @ Wx.T + bx)
    at = exp(-(C*softplus(Lambda)) * rt)
    ht = at * ht_minus_1 + sqrt(1 - at^2) * (it * xt)

Sharding: data-parallel over the batch dim across 8 NeuronCores; weights
replicated.  Compute runs in a transposed layout ([D, B] with D on the
partition axis) so the per-feature vectors (ba, bx, -C*softplus(Lambda))
ride in the ACT engine's per-partition scale/bias operands, and xt is
already K-major for the PE.

Matmuls and element-wise intermediates run in bf16 (fp32 PSUM
accumulation, fp32 output): bf16 matmul streams at full PE rate while
fp32 runs at 1/4, and bf16 doubles DVE throughput.  sqrt(1-at^2) is
computed as exp(0.5*ln(1-at^2)) on wide group tiles; ACT instructions
are chained in emission order (sync=False deps) so the scheduler cannot
interleave different ACT table sets — every alternation would cost a
~1.5us ACT_TABLE_LOAD.
"""

import sys

if "/opt/trn_rl_repo" not in sys.path:
    sys.path.insert(0, "/opt/trn_rl_repo")

import numpy as np

B, D = 16384, 1024
C = 8.0
NCORES = 8
BS = B // NCORES          # 2048 batch rows per core
PT = 128                  # partition tile
KT = D // PT              # 8 k-tiles (contraction)
JT = D // PT              # 8 j-tiles (output features)
CHUNKS = (896, 896, 256)  # batch-chunk widths per core (sum == BS)
# last chunk is small to shrink the post-matmul phase-2 tail

_CACHE = {}


def _build():
    from contextlib import ExitStack

    import concourse.mybir as mybir
    import concourse.tile as tile
    from concourse.tile import add_dep_helper
    from concourse import bacc

    f32 = mybir.dt.float32
    bf16 = mybir.dt.bfloat16
    AF = mybir.ActivationFunctionType
    ALU = mybir.AluOpType
    LNHALF = float(np.log(0.5))

    nc = bacc.Bacc("TRN2", target_bir_lowering=False, debug=False,
                   num_devices=NCORES, dynamic_dma_scratch_size=4096)

    xtT = nc.dram_tensor("xtT", [D, BS], bf16, kind="ExternalInput").ap()
    htT = nc.dram_tensor("htT", [D, BS], bf16, kind="ExternalInput").ap()
    waT = nc.dram_tensor("waT", [D, D], bf16, kind="ExternalInput").ap()
    wxT = nc.dram_tensor("wxT", [D, D], bf16, kind="ExternalInput").ap()
    # consts[:, 0:JT] = ba/2, [:, JT:2JT] = bx/2, [:, 2JT:3JT] = negk/2,
    # [:, 3JT] = ln(0.5)
    consts = nc.dram_tensor("consts", [PT, 3 * JT + 1], f32,
                            kind="ExternalInput").ap()
    outT = nc.dram_tensor("outT", [D, BS], f32, kind="ExternalOutput").ap()

    ln_ops = [[] for _ in CHUNKS]  # natural-log-set ACT ops per chunk

    with tile.TileContext(nc) as tc, ExitStack() as ctx:
        wpool = ctx.enter_context(tc.tile_pool(name="w", bufs=1))
        cpool = ctx.enter_context(tc.tile_pool(name="c", bufs=1))
        xpool = ctx.enter_context(tc.tile_pool(name="x", bufs=2))
        gpool = ctx.enter_context(tc.tile_pool(name="g", bufs=2))
        tpool = ctx.enter_context(tc.tile_pool(name="t", bufs=1))
        pzpool = ctx.enter_context(tc.tile_pool(name="pz", bufs=2, space="PSUM"))

        # DMA order: chunk-0 x, Wa, consts, Wx — first accumulation group
        # starts after ~x+Wa; biases are only needed at the first tanh.
        Q0 = CHUNKS[0]
        x_g0 = xpool.tile([PT, KT, Q0], bf16, name="xg0", tag="x")
        nc.sync.dma_start(out=x_g0,
                          in_=xtT[:, 0:Q0].rearrange("(kt p) q -> p kt q", p=PT))
        wa_g = wpool.tile([PT, KT, D], bf16, name="wag", tag="wa")
        nc.sync.dma_start(out=wa_g,
                          in_=waT.rearrange("(kt p) j -> p kt j", p=PT))
        c_sb = cpool.tile([PT, 3 * JT + 1], f32, tag="c")
        nc.sync.dma_start(out=c_sb, in_=consts)
        ba2_sb = c_sb[:, 0:JT]
        bx2_sb = c_sb[:, JT:2 * JT]
        nk2_sb = c_sb[:, 2 * JT:3 * JT]
        lnhalf_sb = c_sb[:, 3 * JT:3 * JT + 1]
        wx_g = wpool.tile([PT, KT, D], bf16, name="wxg", tag="wx")
        nc.sync.dma_start(out=wx_g,
                          in_=wxT.rearrange("(kt p) j -> p kt j", p=PT))

        x_next = {1: None}
        coff = 0
        x_g = x_g0
        for ci, Q in enumerate(CHUNKS):
            bsl = slice(coff, coff + Q)
            coff += Q
            nsls = []
            off = 0
            while off < Q:
                w = min(512, Q - off)
                nsls.append(slice(off, off + w))
                off += w

            rt_g = gpool.tile([PT, JT, Q], bf16, tag="rt", name=f"rt{ci}")
            p_g = gpool.tile([PT, JT, Q], bf16, tag="p", name=f"p{ci}")

            # ---- phase 1: GEMMs (bf16, fp32 PSUM); Ta/Tx tanh halves ----
            # rt = 0.5 + 0.5*tanh(za/2 + ba/2); tanh shares the ACT table
            # set with exp, so phase 1 and phase 2 never thrash sets.
            for j in range(JT):
                jsl = slice(j * PT, (j + 1) * PT)
                za = pzpool.tile([PT, Q], f32, tag="za", name=f"za{ci}_{j}")
                zx = pzpool.tile([PT, Q], f32, tag="zx", name=f"zx{ci}_{j}")
                for k in range(KT):
                    for nsl in nsls:
                        nc.tensor.matmul(za[:, nsl], wa_g[:, k, jsl],
                                         x_g[:, k, nsl],
                                         start=(k == 0), stop=(k == KT - 1))
                for k in range(KT):
                    for nsl in nsls:
                        nc.tensor.matmul(zx[:, nsl], wx_g[:, k, jsl],
                                         x_g[:, k, nsl],
                                         start=(k == 0), stop=(k == KT - 1))
                nc.scalar.activation(out=rt_g[:, j, :], in_=za, func=AF.Tanh,
                                     bias=ba2_sb[:, j:j + 1], scale=0.5)
                nc.scalar.activation(out=p_g[:, j, :], in_=zx, func=AF.Tanh,
                                     bias=bx2_sb[:, j:j + 1], scale=0.5)
                # p' = (Tx + 1) * x  (= 2*it*xt; the 1/2 folds into s')
                nc.vector.scalar_tensor_tensor(
                    out=p_g[:, j, :], in0=p_g[:, j, :], scalar=1.0,
                    in1=x_g[:, j, :], op0=ALU.add, op1=ALU.mult)

            # Prefetch next chunk's x before this chunk's h/out DMAs enter
            # the sync queue (HWDGE is FIFO per engine).
            if ci + 1 < len(CHUNKS):
                Qn = CHUNKS[ci + 1]
                nbsl = slice(coff, coff + Qn)
                x_g = xpool.tile([PT, KT, Qn], bf16, name=f"xg{ci+1}", tag="x")
                nc.sync.dma_start(
                    out=x_g,
                    in_=xtT[:, nbsl].rearrange("(kt p) q -> p kt q", p=PT))

            # ---- phase 2 ----
            # rt <- negk/2 * Ta + negk/2  (== negk * rt), then exp -> at
            for j in range(JT):
                nc.vector.tensor_scalar(
                    out=rt_g[:, j, :], in0=rt_g[:, j, :],
                    scalar1=nk2_sb[:, j:j + 1], scalar2=nk2_sb[:, j:j + 1],
                    op0=ALU.mult, op1=ALU.add)

            h_g = tpool.tile([PT, JT, Q], bf16, tag="h", name=f"h{ci}")
            halves = [slice(0, JT // 2), slice(JT // 2, JT)]
            for hs in halves:
                nc.sync.dma_start(
                    out=h_g[:, hs, :],
                    in_=htT[hs.start * PT:hs.stop * PT, bsl].rearrange(
                        "(jt p) q -> p jt q", p=PT))

            a2_g = gpool.tile([PT, JT, Q], bf16, tag="a2", name=f"a2{ci}")
            m1_g = gpool.tile([PT, JT, Q], bf16, tag="m1", bufs=1,
                              name=f"m1{ci}")
            o_g = gpool.tile([PT, JT, Q], f32, tag="o", bufs=1, name=f"o{ci}")

            for hs in halves:
                nc.scalar.activation(out=rt_g[:, hs, :], in_=rt_g[:, hs, :],
                                     func=AF.Exp)
            for j in range(JT):
                nc.vector.tensor_mul(a2_g[:, j, :], rt_g[:, j, :],
                                     rt_g[:, j, :])
                nc.vector.tensor_mul(m1_g[:, j, :], rt_g[:, j, :],
                                     h_g[:, j, :])
            for hs in halves:
                ln_ops[ci].append(
                    nc.scalar.activation(out=a2_g[:, hs, :],
                                         in_=a2_g[:, hs, :], func=AF.Ln,
                                         bias=1.0, scale=-1.0))
            for hs in halves:
                # s' = exp(0.5*ln(1-at^2) + ln(0.5)) = sqrt(1-at^2)/2
                nc.scalar.activation(out=a2_g[:, hs, :], in_=a2_g[:, hs, :],
                                     func=AF.Exp, scale=0.5, bias=lnhalf_sb)
            for hs in halves:
                for j in range(hs.start, hs.stop):
                    nc.vector.tensor_mul(p_g[:, j, :], a2_g[:, j, :],
                                         p_g[:, j, :])
                    nc.vector.tensor_add(o_g[:, j, :], m1_g[:, j, :],
                                         p_g[:, j, :])
                nc.sync.dma_start(
                    out=outT[hs.start * PT:hs.stop * PT, bsl].rearrange(
                        "(jt p) q -> p jt q", p=PT),
                    in_=o_g[:, hs, :])

        # Keep the two Ln halves of a chunk adjacent on the ACT stream so
        # the natural-log table set loads once per chunk.
        for ops in ln_ops:
            for a, b in zip(ops, ops[1:]):
                add_dep_helper(b.ins, a.ins, sync=False, reason="ln adjacency")

    nc.compile()
    return nc


def _np_softplus(x):
    return np.logaddexp(0.0, x)


def _fold(vec):
    # [D] feature vector -> [128, JT] tile where column j holds features
    # j*128 .. j*128+127 (per-partition scalars for j-tile j).
    return np.ascontiguousarray(vec.reshape(JT, PT).T)


def _prep(xt, ht, Wa, Wx, ba, bx, Lam):
    import ml_dtypes

    bf16 = ml_dtypes.bfloat16
    negk_vec = (-C * _np_softplus(Lam.astype(np.float64))).astype(np.float32)
    xtT = np.ascontiguousarray(xt.T.astype(bf16))
    htT = np.ascontiguousarray(ht.T.astype(bf16))
    waT = np.ascontiguousarray(Wa.T.astype(bf16))
    wxT = np.ascontiguousarray(Wx.T.astype(bf16))
    consts = np.concatenate(
        [_fold(0.5 * ba), _fold(0.5 * bx), _fold(0.5 * negk_vec),
         np.full((PT, 1), np.log(0.5), np.float32)], axis=1)
    consts = np.ascontiguousarray(consts)
    in_maps = []
    for c in range(NCORES):
        sl = slice(c * BS, (c + 1) * BS)
        in_maps.append({
            "xtT": np.ascontiguousarray(xtT[:, sl]),
            "htT": np.ascontiguousarray(htT[:, sl]),
            "waT": waT,
            "wxT": wxT,
            "consts": consts,
        })
    return in_maps


def kernel(xt, ht_minus_1, Wa, Wx, ba, bx, Lambda):
    from concourse.bass_utils import run_bass_kernel_spmd

    if "nc" not in _CACHE:
        _CACHE["nc"] = _build()
    nc = _CACHE["nc"]

    in_maps = _prep(
        np.asarray(xt, dtype=np.float32),
        np.asarray(ht_minus_1, dtype=np.float32),
        np.asarray(Wa, dtype=np.float32),
        np.asarray(Wx, dtype=np.float32),
        np.asarray(ba, dtype=np.float32).reshape(-1),
        np.asarray(bx, dtype=np.float32).reshape(-1),
        np.asarray(Lambda, dtype=np.float32).reshape(-1),
    )
    res = run_bass_kernel_spmd(nc, in_maps, list(range(NCORES)))
    outT = np.concatenate([res.results[c]["outT"] for c in range(NCORES)],
                          axis=1)
    return np.ascontiguousarray(outT.T)
